# revision 1
# baseline (speedup 1.0000x reference)
"""Trainium2 Bass kernel for sum-of-7-box-blurs (k=3..15, edge padding) * base_map.

Math: out = base_map * sum_k 1/(7 k^2) * V_k(H_k(x)) with V_k/H_k k-wide box
sums (edge padding = clamped indexing, handled by host-side padding).

Horizontal delta decomposition (p = (j-1)/2, d_j = x<<p + x>>p column shifts):
  acc = M_3 x + sum_{j in 3..15 step 2} M_j d_j,   M_j = sum_{k>=j} c_k A_k
where M_j are 15-wide banded vertical matrices folded on the host. Per out
row-tile the vertical mix is a K=128 bf16 matmul accumulating in fp32 PSUM.
d3/d11/d15 are materialized on DVE (even col offsets -> bf16 2x mode), d7/d9
on GPSIMD, and x/d5/d13 are fed as direct column-shifted matmuls (shifts are
free in the rhs access pattern). Matmuls are issued weight-major across each
2048-col window so the PE amortizes weight loads and stays dense (HAM warm).
ScalarE evacuates PSUM; DVE does the base_map multiply in SBUF.

Sharding: rows split across 8 cores; halo rows come from host-side edge
padding so cores are fully independent.
"""

import numpy as np
import ml_dtypes

import concourse.bass as bass
import concourse.mybir as mybir
import concourse.tile as tile
from concourse import bacc, bass_utils

H = W = 4096
NC = 8
RPC = H // NC                 # 512 output rows per core
PAD = 7
PW = W + 2 * PAD              # 4110 padded cols
PR = RPC + 2 * PAD            # 526 padded rows per core
M_TILE = 114                  # valid out rows per PE tile (114 + 14 = 128)
ROW_TILES = [(0, 114), (114, 114), (228, 114), (342, 114), (456, 56)]
CHUNK = 2048                  # column window for arrays + weight-major matmuls
K_SIZES = [3, 5, 7, 9, 11, 13, 15]
BF16 = mybir.dt.bfloat16
F32 = mybir.dt.float32
NP_BF16 = ml_dtypes.bfloat16


def _weights_np() -> np.ndarray:
    """lhsT matrices [7, 128, 128]: lhsT[j][i, m] = w_j[i - m].

    m >= M_TILE columns produce partial sums for out-of-tile rows; they are
    never read. Full 128 weight columns enable fast weight load (FWL)."""
    c = {k: 1.0 / (len(K_SIZES) * k * k) for k in K_SIZES}
    wts = np.zeros((7, 128 + 2 * PAD, 128), dtype=np.float64)
    for ji, j in enumerate(K_SIZES):
        w = np.array(
            [sum(c[k] for k in K_SIZES if k >= j and k >= 2 * abs(d - PAD) + 1)
             for d in range(2 * PAD + 1)])
        for m in range(128):
            wts[ji, m:m + 15, m] = w
    return wts[:, :128, :].astype(NP_BF16)


def _kernel_body(nc, tc, xp_d, bm_d, w_d, out_d):
    add = mybir.AluOpType.add
    mult = mybir.AluOpType.mult

    with (
        tc.tile_pool(name="wpool", bufs=1) as wpool,
        tc.tile_pool(name="xpool", bufs=3) as xpool,
        tc.tile_pool(name="apool", bufs=2) as apool,
        tc.tile_pool(name="bmpool", bufs=3) as bmpool,
        tc.tile_pool(name="ppool", bufs=2) as ppool,
        tc.tile_pool(name="opool", bufs=3) as opool,
        tc.tile_pool(name="psum", bufs=4, space="PSUM") as psum_pool,
    ):
        wsb = wpool.tile([128, 7 * 128], BF16)
        nc.sync.dma_start(
            out=wsb.rearrange("k (j m) -> k j m", j=7),
            in_=w_d.rearrange("j k m -> k j m"))

        def wt(ji, Krows):
            return wsb[:Krows, ji * 128:(ji + 1) * 128]

        # PE warmup: keep the HAM activity window busy during the initial
        # DMA fill so real matmuls start at full clock.
        warm = [psum_pool.tile([128, 1024], F32, tag="ps", name=f"warm{i}") for i in range(2)]
        for i in range(64):
            s = i % 4
            nc.tensor.matmul(
                warm[s // 2][:, (s % 2) * 512:(s % 2 + 1) * 512],
                wsb[:, 0:128], wsb[:, 128:640],
                start=(i < 4), stop=(i >= 60))

        def load_tile(rt, Mt):
            Krows = min(128, PR - rt)
            x_sb = xpool.tile([128, PW], BF16, tag="x")
            nc.sync.dma_start(out=x_sb[:Krows], in_=xp_d[rt:rt + Krows])
            bm_sb = bmpool.tile([128, W], F32, tag="bm")
            nc.sync.dma_start(out=bm_sb[:Mt], in_=bm_d[rt:rt + Mt])
            return x_sb, bm_sb

        loaded = [load_tile(*ROW_TILES[0]), load_tile(*ROW_TILES[1])]
        for ri, (rt, Mt) in enumerate(ROW_TILES):
            Krows = min(128, PR - rt)     # 128, last tile 70
            x_sb, bm_sb = loaded[ri]
            if ri + 2 < len(ROW_TILES):
                loaded.append(load_tile(*ROW_TILES[ri + 2]))
            X = x_sb[:Krows]

            for co in range(0, W, CHUNK):
                # materialized delta arrays (even col offsets -> DVE 2x mode)
                d3 = apool.tile([128, CHUNK], BF16, tag="d3")
                d11 = apool.tile([128, CHUNK], BF16, tag="d11")
                d15 = apool.tile([128, CHUNK], BF16, tag="d15")
                d7 = apool.tile([128, CHUNK], BF16, tag="d7", bufs=4)
                d9 = apool.tile([128, CHUNK], BF16, tag="d9")
                nc.vector.tensor_tensor(
                    out=d3[:Krows], in0=X[:, co + 6:co + 6 + CHUNK],
                    in1=X[:, co + 8:co + 8 + CHUNK], op=add)
                nc.vector.tensor_tensor(
                    out=d11[:Krows], in0=X[:, co + 2:co + 2 + CHUNK],
                    in1=X[:, co + 12:co + 12 + CHUNK], op=add)
                nc.vector.tensor_tensor(
                    out=d15[:Krows], in0=X[:, co + 0:co + 0 + CHUNK],
                    in1=X[:, co + 14:co + 14 + CHUNK], op=add)
                nc.gpsimd.tensor_tensor(
                    out=d7[:Krows], in0=X[:, co + 4:co + 4 + CHUNK],
                    in1=X[:, co + 10:co + 10 + CHUNK], op=add)
                nc.vector.tensor_tensor(
                    out=d9[:Krows], in0=X[:, co + 3:co + 3 + CHUNK],
                    in1=X[:, co + 11:co + 11 + CHUNK], op=add)

                psA = psum_pool.tile([128, 1024], F32, tag="ps")
                psB = psum_pool.tile([128, 1024], F32, tag="ps")
                nsl = CHUNK // 512

                def mms(ji, rhs_of, start=False, stop=False):
                    for s in range(nsl):
                        pt = psA if s < 2 else psB
                        nc.tensor.matmul(
                            pt[:, (s % 2) * 512:(s % 2 + 1) * 512],
                            wt(ji, Krows), rhs_of(s), start=start, stop=stop)

                def xs(s, off):
                    base = co + s * 512 + off
                    return X[:, base:base + 512]

                # weight-major over the window; gpsimd-fed terms last
                mms(0, lambda s: xs(s, 7), start=True)          # x base
                mms(0, lambda s: d3[:Krows, s * 512:s * 512 + 512])
                mms(1, lambda s: xs(s, 5))                      # d5 pair
                mms(1, lambda s: xs(s, 9))
                mms(4, lambda s: d11[:Krows, s * 512:s * 512 + 512])
                mms(5, lambda s: xs(s, 1))                      # d13 pair
                mms(5, lambda s: xs(s, 13))
                mms(6, lambda s: d15[:Krows, s * 512:s * 512 + 512])
                mms(3, lambda s: d9[:Krows, s * 512:s * 512 + 512])
                mms(2, lambda s: d7[:Krows, s * 512:s * 512 + 512],
                    stop=True)

                # evacuate PSUM on ScalarE, multiply by base_map on DVE
                for hi, pt in ((0, psA), (1, psB)):
                    oc = co + hi * 1024
                    psc = ppool.tile([128, 1024], F32, tag="psc")
                    nc.scalar.copy(out=psc[:Mt], in_=pt[:Mt])
                    osb = opool.tile([128, 1024], F32, tag="o")
                    nc.vector.tensor_tensor(
                        out=osb[:Mt], in0=psc[:Mt],
                        in1=bm_sb[:Mt, oc:oc + 1024], op=mult)
                    nc.sync.dma_start(
                        out=out_d[rt:rt + Mt, oc:oc + 1024], in_=osb[:Mt])


def _build():
    nc = bacc.Bacc("TRN2", target_bir_lowering=False, debug=False)
    xp_d = nc.dram_tensor("xp", [PR, PW], BF16, kind="ExternalInput").ap()
    bm_d = nc.dram_tensor("bm", [RPC, W], F32, kind="ExternalInput").ap()
    w_d = nc.dram_tensor("wts", [7, 128, 128], BF16, kind="ExternalInput").ap()
    out_d = nc.dram_tensor("out", [RPC, W], F32, kind="ExternalOutput").ap()
    with tile.TileContext(nc) as tc:
        _kernel_body(nc, tc, xp_d, bm_d, w_d, out_d)
    nc.compile()
    return nc


_CACHE: dict = {}


def _get_nc():
    if "nc" not in _CACHE:
        _CACHE["nc"] = _build()
    return _CACHE["nc"]


def _in_maps(x: np.ndarray, base_map: np.ndarray) -> list[dict]:
    xp = np.pad(x, PAD, mode="edge").astype(NP_BF16)
    wts = _weights_np()
    maps = []
    for c in range(NC):
        maps.append({
            "xp": np.ascontiguousarray(xp[c * RPC: c * RPC + PR]),
            "bm": np.ascontiguousarray(base_map[c * RPC:(c + 1) * RPC]),
            "wts": wts,
        })
    return maps


def run(x, base_map, **kwargs) -> tuple[np.ndarray, bass_utils.BassKernelResults]:
    x = np.ascontiguousarray(np.asarray(x), dtype=np.float32)
    base_map = np.ascontiguousarray(np.asarray(base_map), dtype=np.float32)
    nc = _get_nc()
    res = bass_utils.run_bass_kernel_spmd(
        nc, _in_maps(x, base_map), core_ids=list(range(NC)), **kwargs)
    out = np.concatenate([r["out"] for r in res.results], axis=0)
    return out[None, None].astype(np.float32, copy=False), res


def kernel(x, base_map) -> np.ndarray:
    return run(x, base_map)[0]



# revision 8
# speedup vs baseline: 1.5954x; 1.5954x over previous
"""Trainium2 Bass kernel for sum-of-7-box-blurs (k=3..15, edge padding) * base_map.

out = bm * sum_t W_t (x @ col-shift t), t = 0..14, W_t = 15-wide banded vertical
matrices with W_t[i, m] = W2D[i-m, t], W2D[d, t] = f(max(|d-7|,|t-7|)),
f(m) = sum_{k >= 2m+1} 1/(7 k^2).

All 15 horizontal taps are fed DIRECTLY to the PE from fp8(e4m3) copies of x —
no DVE/GPSIMD delta materialization at all. Taps are processed in pairs
(2i, 2i+1) by fp8 DoubleRow matmuls: the rhs 3D AP [K, 2, N] interleaves
phase0 = x and phase1 = x shifted one column (stored as one host-interleaved
HBM array so the phase step is 16B-aligned), and the [K, 2, 128] weights carry
the two taps' bands. 7 DoubleRow + 1 normal fp8 matmul per 512-col window
replace the baseline's 10 bf16 matmuls + 5 elementwise delta arrays.

Weights are scaled by 2048 (fp8 subnormal underflow) and stratified-rounded
(ring-sum-preserving) to kill systematic quantization bias; the 1/2048 is
applied by ScalarE during PSUM evacuation. base_map and out are fp16.

Sharding: rows split across 8 cores; halo rows come from host-side edge
padding so cores are fully independent.
"""

import numpy as np
import ml_dtypes

import concourse.bass as bass
import concourse.mybir as mybir
import concourse.tile as tile
from concourse import bacc, bass_utils

H = W = 4096
NC = 8
RPC = H // NC                 # 512 output rows per core
PAD = 7
PR = RPC + 2 * PAD            # 526 padded rows per core
PWA = 4112                    # padded cols per phase (16B-aligned phase step)
CHUNK = 2048
ROW_TILES = [(0, 114), (114, 114), (228, 114), (342, 114), (456, 56)]
WSCALE = 2048.0
K_SIZES = [3, 5, 7, 9, 11, 13, 15]
BF8 = mybir.dt.float8e4
FP16 = mybir.dt.float16
F32 = mybir.dt.float32
NP_E4M3 = ml_dtypes.float8_e4m3
DR = mybir.MatmulPerfMode.DoubleRow


def _w2d() -> np.ndarray:
    c = {k: 1.0 / (len(K_SIZES) * k * k) for k in K_SIZES}
    f = [sum(c[k] for k in K_SIZES if k >= 2 * m + 1) for m in range(8)]
    Wt = np.zeros((15, 15))
    for d in range(15):
        for t in range(15):
            Wt[d, t] = f[max(abs(d - 7), abs(t - 7))]
    return Wt


def _stratified_quant(Wt: np.ndarray, seed=0) -> np.ndarray:
    """Round W*WSCALE to e4m3 neighbors; per ring of equal values pick the
    round-up count that preserves the ring sum, spread randomly."""
    Ws = (Wt * WSCALE).astype(np.float64)
    grid = np.unique(
        np.arange(256, dtype=np.uint8).view(NP_E4M3).astype(np.float64))
    grid = grid[np.isfinite(grid)]
    gi = np.searchsorted(grid, Ws)
    a = grid[np.clip(gi - 1, 0, None)]
    b = grid[gi]
    exact = np.isclose(a, Ws)
    a = np.where(exact, Ws, a)
    b = np.where(exact, Ws, b)
    out = np.zeros_like(Ws)
    rng = np.random.default_rng(seed)
    for v in np.unique(Ws):
        mask = Ws == v
        n = int(mask.sum())
        av, bv = a[mask][0], b[mask][0]
        if bv == av:
            out[mask] = av
            continue
        k = int(round((v - av) / (bv - av) * n))
        picks = np.zeros(n, dtype=bool)
        picks[rng.permutation(n)[:k]] = True
        out[mask] = np.where(picks, bv, av)
    return out


def _weights_np():
    """Pair weights [7, 128, 2, 128] (taps 2i, 2i+1) + single [128, 128]
    (tap 14), e4m3, scaled by WSCALE. wp[i][k, ph, m] = W2D[k-m, 2i+ph]."""
    Wq = _stratified_quant(_w2d())
    band = np.zeros((15, 128 + 15, 128))
    for t in range(15):
        for m in range(128):
            band[t, m:m + 15, m] = Wq[:, t]
    band = band[:, :128, :]
    wp = np.stack([np.stack([band[2 * i], band[2 * i + 1]], axis=1)
                   for i in range(7)])
    return wp.astype(NP_E4M3), band[14].astype(NP_E4M3)


def _kernel_body(nc, tc, xab_d, bm_d, wp_d, ws_d, out_d):
    mult = mybir.AluOpType.mult

    with (
        tc.tile_pool(name="wpool", bufs=1) as wpool,
        tc.tile_pool(name="xpool", bufs=3) as xpool,
        tc.tile_pool(name="bmpool", bufs=3) as bmpool,
        tc.tile_pool(name="ppool", bufs=4) as ppool,
        tc.tile_pool(name="opool", bufs=4) as opool,
        tc.tile_pool(name="psum", bufs=8, space="PSUM") as psum_pool,
    ):
        # DoubleRow LDWEIGHTS needs each pair weight in its own narrow tile
        # (offset views into a wide tile load garbage weights).
        wtiles = []
        for i in range(7):
            wt = wpool.tile([128, 256], BF8, name=f"wp{i}")
            nc.sync.dma_start(
                out=wt.rearrange("k (two m) -> k two m", two=2), in_=wp_d[i])
            wtiles.append(wt.rearrange("k (two m) -> k two m", two=2))
        wssb = wpool.tile([128, 128], BF8)
        nc.sync.dma_start(out=wssb, in_=ws_d)

        # PE warmup: keep the HAM activity window busy during the initial
        # DMA fill so real matmuls start at full clock.
        warm = [psum_pool.tile([128, 512], F32, tag="ps", name=f"warm{i}")
                for i in range(2)]
        for i in range(48):
            nc.tensor.matmul(
                warm[(i % 4) // 2][:, :128], wssb, wssb,
                start=(i < 4), stop=(i >= 44))

        def load_tile(rt, Mt):
            Krows = min(128, PR - rt)
            x_sb = xpool.tile([128, 2 * PWA], BF8, tag="x")
            nc.sync.dma_start(
                out=x_sb.rearrange("k (two w) -> k two w", two=2)[:Krows],
                in_=xab_d[rt:rt + Krows])
            bm_sb = bmpool.tile([128, W], FP16, tag="bm")
            nc.sync.dma_start(out=bm_sb[:Mt], in_=bm_d[rt:rt + Mt])
            return x_sb, bm_sb

        loaded = [load_tile(*ROW_TILES[0]), load_tile(*ROW_TILES[1])]
        for ri, (rt, Mt) in enumerate(ROW_TILES):
            Krows = min(128, PR - rt)     # 128, last tile 70
            x_sb, bm_sb = loaded[ri]
            if ri + 2 < len(ROW_TILES):
                loaded.append(load_tile(*ROW_TILES[ri + 2]))
            X3 = x_sb.rearrange("k (two w) -> k two w", two=2)

            for co in range(0, W, CHUNK):
                pss = [psum_pool.tile([128, 512], F32, tag="ps",
                                      name=f"ps{ri}_{co}_{s}")
                       for s in range(4)]
                # weight-major: each pair weight streams all 4 windows
                for i in range(7):
                    for s in range(4):
                        base = co + s * 512 + 2 * i
                        nc.tensor.matmul(
                            pss[s], wtiles[i][:Krows],
                            X3[:Krows, :, base:base + 512],
                            start=(i == 0), stop=False, perf_mode=DR)
                for s in range(4):
                    base = co + s * 512 + 14
                    nc.tensor.matmul(
                        pss[s], wssb[:Krows],
                        X3[:Krows, 0, base:base + 512],
                        start=False, stop=True)

                for s in range(4):
                    oc = co + s * 512
                    psc = ppool.tile([128, 512], FP16, tag="psc")
                    nc.scalar.mul(psc[:Mt], pss[s][:Mt], 1.0 / WSCALE)
                    osb = opool.tile([128, 512], FP16, tag="o")
                    nc.vector.tensor_tensor(
                        out=osb[:Mt], in0=psc[:Mt],
                        in1=bm_sb[:Mt, oc:oc + 512], op=mult)
                    nc.sync.dma_start(
                        out=out_d[rt:rt + Mt, oc:oc + 512], in_=osb[:Mt])


def _build():
    nc = bacc.Bacc("TRN2", target_bir_lowering=False, debug=False)
    xab_d = nc.dram_tensor("xab", [PR, 2, PWA], BF8, kind="ExternalInput").ap()
    bm_d = nc.dram_tensor("bm", [RPC, W], FP16, kind="ExternalInput").ap()
    wp_d = nc.dram_tensor("wp", [7, 128, 2, 128], BF8, kind="ExternalInput").ap()
    ws_d = nc.dram_tensor("ws", [128, 128], BF8, kind="ExternalInput").ap()
    out_d = nc.dram_tensor("out", [RPC, W], FP16, kind="ExternalOutput").ap()
    with tile.TileContext(nc) as tc:
        _kernel_body(nc, tc, xab_d, bm_d, wp_d, ws_d, out_d)
    nc.compile()
    return nc


_CACHE: dict = {}


def _get_nc():
    if "nc" not in _CACHE:
        _CACHE["nc"] = _build()
    return _CACHE["nc"]


def _in_maps(x: np.ndarray, base_map: np.ndarray) -> list[dict]:
    xq = np.pad(x, ((PAD, PAD), (PAD, PWA + 1 - W - PAD)),
                mode="edge").astype(NP_E4M3)
    xab = np.stack([xq[:, 0:PWA], xq[:, 1:PWA + 1]], axis=1)
    bmh = base_map.astype(np.float16)
    wp, ws = _weights_np()
    maps = []
    for c in range(NC):
        maps.append({
            "xab": np.ascontiguousarray(xab[c * RPC: c * RPC + PR]),
            "bm": np.ascontiguousarray(bmh[c * RPC:(c + 1) * RPC]),
            "wp": wp,
            "ws": ws,
        })
    return maps


def run(x, base_map, **kwargs) -> tuple[np.ndarray, bass_utils.BassKernelResults]:
    x = np.ascontiguousarray(np.asarray(x), dtype=np.float32)
    base_map = np.ascontiguousarray(np.asarray(base_map), dtype=np.float32)
    nc = _get_nc()
    res = bass_utils.run_bass_kernel_spmd(
        nc, _in_maps(x, base_map), core_ids=list(range(NC)), **kwargs)
    out = np.concatenate([r["out"] for r in res.results], axis=0)
    return out[None, None].astype(np.float32), res


def kernel(x, base_map) -> np.ndarray:
    return run(x, base_map)[0]


# revision 11
# speedup vs baseline: 1.6514x; 1.0351x over previous
"""Trainium2 Bass kernel for sum-of-7-box-blurs (k=3..15, edge padding) * base_map.

out = bm * sum_t W_t (x @ col-shift t), t = 0..14, W_t = 15-wide banded vertical
matrices with W_t[i, m] = W2D[i-m, t], W2D[d, t] = f(max(|d-7|,|t-7|)),
f(m) = sum_{k >= 2m+1} 1/(7 k^2).

All 15 horizontal taps are fed DIRECTLY to the PE from fp8(e4m3) copies of x —
no DVE/GPSIMD delta materialization at all. Taps are processed in pairs
(2i, 2i+1) by fp8 DoubleRow matmuls: the rhs 3D AP [K, 2, N] interleaves
phase0 = x and phase1 = x shifted one column (stored as one host-interleaved
HBM array so the phase step is 16B-aligned), and the [K, 2, 128] weights carry
the two taps' bands. 7 DoubleRow + 1 normal fp8 matmul per 512-col window
replace the baseline's 10 bf16 matmuls + 5 elementwise delta arrays.

Weights are scaled by 2048 (fp8 subnormal underflow) and stratified-rounded
(ring-sum-preserving) to kill systematic quantization bias; the 1/2048 is
applied by ScalarE during PSUM evacuation. base_map and out are fp16.

Sharding: rows split across 8 cores; halo rows come from host-side edge
padding so cores are fully independent.
"""

import numpy as np
import ml_dtypes

import concourse.bass as bass
import concourse.mybir as mybir
import concourse.tile as tile
from concourse import bacc, bass_utils

H = W = 4096
NC = 8
RPC = H // NC                 # 512 output rows per core
PAD = 7
PR = RPC + 2 * PAD            # 526 padded rows per core
PWA = 4112                    # padded cols per phase (16B-aligned phase step)
CHUNK = 2048
ROW_TILES = [(0, 114), (114, 114), (228, 114), (342, 114), (456, 56)]
WSCALE = 2048.0
K_SIZES = [3, 5, 7, 9, 11, 13, 15]
BF8 = mybir.dt.float8e4
FP16 = mybir.dt.float16
F32 = mybir.dt.float32
NP_E4M3 = ml_dtypes.float8_e4m3
DR = mybir.MatmulPerfMode.DoubleRow


def _w2d() -> np.ndarray:
    c = {k: 1.0 / (len(K_SIZES) * k * k) for k in K_SIZES}
    f = [sum(c[k] for k in K_SIZES if k >= 2 * m + 1) for m in range(8)]
    Wt = np.zeros((15, 15))
    for d in range(15):
        for t in range(15):
            Wt[d, t] = f[max(abs(d - 7), abs(t - 7))]
    return Wt


def _stratified_quant(Wt: np.ndarray, seed=0) -> np.ndarray:
    """Round W*WSCALE to e4m3 neighbors; per ring of equal values pick the
    round-up count that preserves the ring sum, spread randomly."""
    Ws = (Wt * WSCALE).astype(np.float64)
    grid = np.unique(
        np.arange(256, dtype=np.uint8).view(NP_E4M3).astype(np.float64))
    grid = grid[np.isfinite(grid)]
    gi = np.searchsorted(grid, Ws)
    a = grid[np.clip(gi - 1, 0, None)]
    b = grid[gi]
    exact = np.isclose(a, Ws)
    a = np.where(exact, Ws, a)
    b = np.where(exact, Ws, b)
    out = np.zeros_like(Ws)
    rng = np.random.default_rng(seed)
    for v in np.unique(Ws):
        mask = Ws == v
        n = int(mask.sum())
        av, bv = a[mask][0], b[mask][0]
        if bv == av:
            out[mask] = av
            continue
        k = int(round((v - av) / (bv - av) * n))
        picks = np.zeros(n, dtype=bool)
        picks[rng.permutation(n)[:k]] = True
        out[mask] = np.where(picks, bv, av)
    return out


def _weights_np():
    """Pair weights [7, 128, 2, 128] (taps 2i, 2i+1) + single [128, 128]
    (tap 14), e4m3, scaled by WSCALE. wp[i][k, ph, m] = W2D[k-m, 2i+ph]."""
    Wq = _stratified_quant(_w2d())
    band = np.zeros((15, 128 + 15, 128))
    for t in range(15):
        for m in range(128):
            band[t, m:m + 15, m] = Wq[:, t]
    band = band[:, :128, :]
    wp = np.stack([np.stack([band[2 * i], band[2 * i + 1]], axis=1)
                   for i in range(7)])
    return wp.astype(NP_E4M3), band[14].astype(NP_E4M3)


def _kernel_body(nc, tc, xab_d, bm_d, wp_d, ws_d, out_d):
    mult = mybir.AluOpType.mult

    with (
        tc.tile_pool(name="wpool", bufs=1) as wpool,
        tc.tile_pool(name="xpool", bufs=3) as xpool,
        tc.tile_pool(name="bmpool", bufs=3) as bmpool,
        tc.tile_pool(name="ppool", bufs=4) as ppool,
        tc.tile_pool(name="opool", bufs=3) as opool,
        tc.tile_pool(name="psum", bufs=8, space="PSUM") as psum_pool,
    ):
        # DoubleRow LDWEIGHTS needs each pair weight in its own narrow tile
        # (offset views into a wide tile load garbage weights).
        wtiles = []
        for i in range(7):
            wt = wpool.tile([128, 256], BF8, name=f"wp{i}")
            nc.sync.dma_start(
                out=wt.rearrange("k (two m) -> k two m", two=2), in_=wp_d[i])
            wtiles.append(wt.rearrange("k (two m) -> k two m", two=2))
        wssb = wpool.tile([128, 128], BF8)
        nc.sync.dma_start(out=wssb, in_=ws_d)

        # PE warmup: keep the HAM activity window busy during the initial
        # DMA fill so real matmuls start at full clock.
        warm = [psum_pool.tile([128, 512], F32, tag="ps", name=f"warm{i}")
                for i in range(2)]
        for i in range(48):
            nc.tensor.matmul(
                warm[(i % 4) // 2][:, :128], wssb, wssb,
                start=(i < 4), stop=(i >= 44))

        def load_tile(rt, Mt):
            Krows = min(128, PR - rt)
            x_sb = xpool.tile([128, 2 * PWA], BF8, tag="x")
            nc.sync.dma_start(
                out=x_sb.rearrange("k (two w) -> k two w", two=2)[:Krows],
                in_=xab_d[rt:rt + Krows])
            bm_sb = bmpool.tile([128, W], FP16, tag="bm")
            nc.sync.dma_start(out=bm_sb[:Mt], in_=bm_d[rt:rt + Mt])
            return x_sb, bm_sb

        loaded = [load_tile(*ROW_TILES[0]), load_tile(*ROW_TILES[1])]
        for ri, (rt, Mt) in enumerate(ROW_TILES):
            Krows = min(128, PR - rt)     # 128, last tile 70
            x_sb, bm_sb = loaded[ri]
            if ri + 2 < len(ROW_TILES):
                loaded.append(load_tile(*ROW_TILES[ri + 2]))
            X3 = x_sb.rearrange("k (two w) -> k two w", two=2)
            osb = opool.tile([128, W], FP16, tag="o")

            for co in range(0, W, CHUNK):
                pss = [psum_pool.tile([128, 512], F32, tag="ps",
                                      name=f"ps{ri}_{co}_{s}")
                       for s in range(4)]
                # weight-major: each pair weight streams all 4 windows
                for i in range(7):
                    for s in range(4):
                        base = co + s * 512 + 2 * i
                        nc.tensor.matmul(
                            pss[s], wtiles[i][:Krows],
                            X3[:Krows, :, base:base + 512],
                            start=(i == 0), stop=False, perf_mode=DR)
                for s in range(4):
                    base = co + s * 512 + 14
                    nc.tensor.matmul(
                        pss[s], wssb[:Krows],
                        X3[:Krows, 0, base:base + 512],
                        start=False, stop=True)

                for s in range(4):
                    oc = co + s * 512
                    psc = ppool.tile([128, 512], FP16, tag="psc")
                    nc.scalar.mul(psc[:Mt], pss[s][:Mt], 1.0 / WSCALE)
                    nc.vector.tensor_tensor(
                        out=osb[:Mt, oc:oc + 512], in0=psc[:Mt],
                        in1=bm_sb[:Mt, oc:oc + 512], op=mult)
            nc.sync.dma_start(out=out_d[rt:rt + Mt], in_=osb[:Mt])


def _build():
    nc = bacc.Bacc("TRN2", target_bir_lowering=False, debug=False)
    xab_d = nc.dram_tensor("xab", [PR, 2, PWA], BF8, kind="ExternalInput").ap()
    bm_d = nc.dram_tensor("bm", [RPC, W], FP16, kind="ExternalInput").ap()
    wp_d = nc.dram_tensor("wp", [7, 128, 2, 128], BF8, kind="ExternalInput").ap()
    ws_d = nc.dram_tensor("ws", [128, 128], BF8, kind="ExternalInput").ap()
    out_d = nc.dram_tensor("out", [RPC, W], FP16, kind="ExternalOutput").ap()
    with tile.TileContext(nc) as tc:
        _kernel_body(nc, tc, xab_d, bm_d, wp_d, ws_d, out_d)
    nc.compile()
    return nc


_CACHE: dict = {}


def _get_nc():
    if "nc" not in _CACHE:
        _CACHE["nc"] = _build()
    return _CACHE["nc"]


def _in_maps(x: np.ndarray, base_map: np.ndarray) -> list[dict]:
    xq = np.pad(x, ((PAD, PAD), (PAD, PWA + 1 - W - PAD)),
                mode="edge").astype(NP_E4M3)
    xab = np.stack([xq[:, 0:PWA], xq[:, 1:PWA + 1]], axis=1)
    bmh = base_map.astype(np.float16)
    wp, ws = _weights_np()
    maps = []
    for c in range(NC):
        maps.append({
            "xab": np.ascontiguousarray(xab[c * RPC: c * RPC + PR]),
            "bm": np.ascontiguousarray(bmh[c * RPC:(c + 1) * RPC]),
            "wp": wp,
            "ws": ws,
        })
    return maps


def run(x, base_map, **kwargs) -> tuple[np.ndarray, bass_utils.BassKernelResults]:
    x = np.ascontiguousarray(np.asarray(x), dtype=np.float32)
    base_map = np.ascontiguousarray(np.asarray(base_map), dtype=np.float32)
    nc = _get_nc()
    res = bass_utils.run_bass_kernel_spmd(
        nc, _in_maps(x, base_map), core_ids=list(range(NC)), **kwargs)
    out = np.concatenate([r["out"] for r in res.results], axis=0)
    return out[None, None].astype(np.float32), res


def kernel(x, base_map) -> np.ndarray:
    return run(x, base_map)[0]


# revision 13
# speedup vs baseline: 1.6693x; 1.0108x over previous
"""Trainium2 Bass kernel for sum-of-7-box-blurs (k=3..15, edge padding) * base_map.

out = bm * sum_t W_t (x @ col-shift t), t = 0..14, W_t = 15-wide banded vertical
matrices with W_t[i, m] = W2D[i-m, t], W2D[d, t] = f(max(|d-7|,|t-7|)),
f(m) = sum_{k >= 2m+1} 1/(7 k^2).

All 15 horizontal taps are fed DIRECTLY to the PE from fp8(e4m3) copies of x —
no DVE/GPSIMD delta materialization at all. Taps are processed in pairs
(2i, 2i+1) by fp8 DoubleRow matmuls: the rhs 3D AP [K, 2, N] interleaves
phase0 = x and phase1 = x shifted one column (stored as one host-interleaved
HBM array so the phase step is 16B-aligned), and the [K, 2, 128] weights carry
the two taps' bands. 7 DoubleRow + 1 normal fp8 matmul per 512-col window
replace the baseline's 10 bf16 matmuls + 5 elementwise delta arrays.

Weights are scaled by 2048 (fp8 subnormal underflow) and stratified-rounded
(ring-sum-preserving) to kill systematic quantization bias; the 1/2048 is
applied by ScalarE during PSUM evacuation. base_map and out are fp16.

Sharding: rows split across 8 cores; halo rows come from host-side edge
padding so cores are fully independent.
"""

import numpy as np
import ml_dtypes

import concourse.bass as bass
import concourse.mybir as mybir
import concourse.tile as tile
from concourse import bacc, bass_utils

H = W = 4096
NC = 8
RPC = H // NC                 # 512 output rows per core
PAD = 7
PR = RPC + 2 * PAD            # 526 padded rows per core
PWA = 4112                    # padded cols per phase (16B-aligned phase step)
CHUNK = 2048
ROW_TILES = [(0, 114), (114, 114), (228, 114), (342, 114), (456, 56)]
WSCALE = 2048.0
K_SIZES = [3, 5, 7, 9, 11, 13, 15]
BF8 = mybir.dt.float8e4
FP16 = mybir.dt.float16
F32 = mybir.dt.float32
NP_E4M3 = ml_dtypes.float8_e4m3
DR = mybir.MatmulPerfMode.DoubleRow


def _w2d() -> np.ndarray:
    c = {k: 1.0 / (len(K_SIZES) * k * k) for k in K_SIZES}
    f = [sum(c[k] for k in K_SIZES if k >= 2 * m + 1) for m in range(8)]
    Wt = np.zeros((15, 15))
    for d in range(15):
        for t in range(15):
            Wt[d, t] = f[max(abs(d - 7), abs(t - 7))]
    return Wt


def _stratified_quant(Wt: np.ndarray, seed=0) -> np.ndarray:
    """Round W*WSCALE to e4m3 neighbors; per ring of equal values pick the
    round-up count that preserves the ring sum, spread randomly."""
    Ws = (Wt * WSCALE).astype(np.float64)
    grid = np.unique(
        np.arange(256, dtype=np.uint8).view(NP_E4M3).astype(np.float64))
    grid = grid[np.isfinite(grid)]
    gi = np.searchsorted(grid, Ws)
    a = grid[np.clip(gi - 1, 0, None)]
    b = grid[gi]
    exact = np.isclose(a, Ws)
    a = np.where(exact, Ws, a)
    b = np.where(exact, Ws, b)
    out = np.zeros_like(Ws)
    rng = np.random.default_rng(seed)
    for v in np.unique(Ws):
        mask = Ws == v
        n = int(mask.sum())
        av, bv = a[mask][0], b[mask][0]
        if bv == av:
            out[mask] = av
            continue
        k = int(round((v - av) / (bv - av) * n))
        picks = np.zeros(n, dtype=bool)
        picks[rng.permutation(n)[:k]] = True
        out[mask] = np.where(picks, bv, av)
    return out


def _weights_np():
    """Pair weights [7, 128, 2, 128] (taps 2i, 2i+1) + single [128, 128]
    (tap 14), e4m3, scaled by WSCALE. wp[i][k, ph, m] = W2D[k-m, 2i+ph]."""
    Wq = _stratified_quant(_w2d())
    band = np.zeros((15, 128 + 15, 128))
    for t in range(15):
        for m in range(128):
            band[t, m:m + 15, m] = Wq[:, t]
    band = band[:, :128, :]
    wp = np.stack([np.stack([band[2 * i], band[2 * i + 1]], axis=1)
                   for i in range(7)])
    return wp.astype(NP_E4M3), band[14].astype(NP_E4M3)


def _kernel_body(nc, tc, xab_d, bm_d, wp_d, ws_d, out_d):
    mult = mybir.AluOpType.mult

    with (
        tc.tile_pool(name="wpool", bufs=1) as wpool,
        tc.tile_pool(name="xpool", bufs=3) as xpool,
        tc.tile_pool(name="bmpool", bufs=3) as bmpool,
        tc.tile_pool(name="ppool", bufs=4) as ppool,
        tc.tile_pool(name="opool", bufs=3) as opool,
        tc.tile_pool(name="psum", bufs=8, space="PSUM") as psum_pool,
    ):
        # DoubleRow LDWEIGHTS needs each pair weight in its own narrow tile
        # (offset views into a wide tile load garbage weights).
        wtiles = []
        for i in range(7):
            wt = wpool.tile([128, 256], BF8, name=f"wp{i}")
            nc.sync.dma_start(
                out=wt.rearrange("k (two m) -> k two m", two=2), in_=wp_d[i])
            wtiles.append(wt.rearrange("k (two m) -> k two m", two=2))
        wssb = wpool.tile([128, 128], BF8)
        nc.sync.dma_start(out=wssb, in_=ws_d)

        # PE warmup on a zeroed dummy tile: starts immediately (no DMA
        # dependency) and spans the initial DMA fill so real matmuls start
        # at full clock.
        wdum = wpool.tile([128, 640], BF8, name="wdum")
        nc.scalar.memzero(wdum)
        warm = [psum_pool.tile([128, 512], F32, tag="ps", name=f"warm{i}")
                for i in range(2)]
        for i in range(16):
            nc.tensor.matmul(
                warm[(i % 4) // 2], wdum[:, :128], wdum[:, 128:640],
                start=(i < 4), stop=(i >= 12))

        def load_tile(rt, Mt):
            Krows = min(128, PR - rt)
            x_sb = xpool.tile([128, 2 * PWA], BF8, tag="x")
            nc.sync.dma_start(
                out=x_sb.rearrange("k (two w) -> k two w", two=2)[:Krows],
                in_=xab_d[rt:rt + Krows])
            bm_sb = bmpool.tile([128, W], FP16, tag="bm")
            nc.sync.dma_start(out=bm_sb[:Mt], in_=bm_d[rt:rt + Mt])
            return x_sb, bm_sb

        loaded = [load_tile(*ROW_TILES[0]), load_tile(*ROW_TILES[1])]
        for ri, (rt, Mt) in enumerate(ROW_TILES):
            Krows = min(128, PR - rt)     # 128, last tile 70
            x_sb, bm_sb = loaded[ri]
            if ri + 2 < len(ROW_TILES):
                loaded.append(load_tile(*ROW_TILES[ri + 2]))
            X3 = x_sb.rearrange("k (two w) -> k two w", two=2)
            osb = opool.tile([128, W], FP16, tag="o")

            for co in range(0, W, CHUNK):
                pss = [psum_pool.tile([128, 512], F32, tag="ps",
                                      name=f"ps{ri}_{co}_{s}")
                       for s in range(4)]
                # weight-major: each pair weight streams all 4 windows
                for i in range(7):
                    for s in range(4):
                        base = co + s * 512 + 2 * i
                        nc.tensor.matmul(
                            pss[s], wtiles[i][:Krows],
                            X3[:Krows, :, base:base + 512],
                            start=(i == 0), stop=False, perf_mode=DR)
                for s in range(4):
                    base = co + s * 512 + 14
                    nc.tensor.matmul(
                        pss[s], wssb[:Krows],
                        X3[:Krows, 0, base:base + 512],
                        start=False, stop=True)

                for s in range(4):
                    oc = co + s * 512
                    psc = ppool.tile([128, 512], FP16, tag="psc")
                    nc.scalar.mul(psc[:Mt], pss[s][:Mt], 1.0 / WSCALE)
                    nc.vector.tensor_tensor(
                        out=osb[:Mt, oc:oc + 512], in0=psc[:Mt],
                        in1=bm_sb[:Mt, oc:oc + 512], op=mult)
                nc.sync.dma_start(
                    out=out_d[rt:rt + Mt, co:co + CHUNK],
                    in_=osb[:Mt, co:co + CHUNK])


def _build():
    nc = bacc.Bacc("TRN2", target_bir_lowering=False, debug=False)
    xab_d = nc.dram_tensor("xab", [PR, 2, PWA], BF8, kind="ExternalInput").ap()
    bm_d = nc.dram_tensor("bm", [RPC, W], FP16, kind="ExternalInput").ap()
    wp_d = nc.dram_tensor("wp", [7, 128, 2, 128], BF8, kind="ExternalInput").ap()
    ws_d = nc.dram_tensor("ws", [128, 128], BF8, kind="ExternalInput").ap()
    out_d = nc.dram_tensor("out", [RPC, W], FP16, kind="ExternalOutput").ap()
    with tile.TileContext(nc) as tc:
        _kernel_body(nc, tc, xab_d, bm_d, wp_d, ws_d, out_d)
    nc.compile()
    return nc


_CACHE: dict = {}


def _get_nc():
    if "nc" not in _CACHE:
        _CACHE["nc"] = _build()
    return _CACHE["nc"]


def _in_maps(x: np.ndarray, base_map: np.ndarray) -> list[dict]:
    xq = np.pad(x, ((PAD, PAD), (PAD, PWA + 1 - W - PAD)),
                mode="edge").astype(NP_E4M3)
    xab = np.stack([xq[:, 0:PWA], xq[:, 1:PWA + 1]], axis=1)
    bmh = base_map.astype(np.float16)
    wp, ws = _weights_np()
    maps = []
    for c in range(NC):
        maps.append({
            "xab": np.ascontiguousarray(xab[c * RPC: c * RPC + PR]),
            "bm": np.ascontiguousarray(bmh[c * RPC:(c + 1) * RPC]),
            "wp": wp,
            "ws": ws,
        })
    return maps


def run(x, base_map, **kwargs) -> tuple[np.ndarray, bass_utils.BassKernelResults]:
    x = np.ascontiguousarray(np.asarray(x), dtype=np.float32)
    base_map = np.ascontiguousarray(np.asarray(base_map), dtype=np.float32)
    nc = _get_nc()
    res = bass_utils.run_bass_kernel_spmd(
        nc, _in_maps(x, base_map), core_ids=list(range(NC)), **kwargs)
    out = np.concatenate([r["out"] for r in res.results], axis=0)
    return out[None, None].astype(np.float32), res


def kernel(x, base_map) -> np.ndarray:
    return run(x, base_map)[0]
